# revision 16
# baseline (speedup 1.0000x reference)
"""CARC attention processor kernel for 8 Trainium2 NeuronCores.

Sharding: data-parallel over the fused B*H axis. 80 heads / 8 cores =
10 heads per core; each core owns one batch (bi = core//2) and one
10-head group (g = core%2). Projection weights are column/row-sliced
per head group; the KV bank is sliced per core. Each core emits a
partial output projection over its 640 channels (fp16); the host sums
the two partials per batch and adds the bias.

Device schedule per core (all matmuls fp16 in, fp32 PSUM):
  - startup, cc-outer over chunk-wise multi-queue hsT DMA: q/k
    projections for pair 0 plus vproj lt0-3 start as soon as the first
    128-row hsT chunk lands (8 PSUM banks: 2+2 proj halves + 4 vproj).
  - attention per pair, per query-half (512 cols): scores are emitted
    as adjacent K=64 matmul pairs at tile_position (0,0)/(64,0) so the
    two heads stream the PE concurrently; one [128,1024] exp (ACT)
    covers both heads; ctx accumulates into 1-bank [128,512] tiles
    (64 ctx rows + 64 ones-denominator rows).
  - remaining vproj tiles and the next pair's q/k projections are
    interleaved into the exp-gated slack of the kc loop.
  - normalization: reciprocal_approx_fast on the denominator rows read
    straight from PSUM, then one tensor_mul into ctxT (fp16).
  - out-projection for query tiles 0-3 overlaps the last pair's second
    half; wo preloads on the idle SP queue during pair 1.
"""
from contextlib import ExitStack

import numpy as np

import concourse.bass as bass
import concourse.tile as tile
from concourse import bacc, mybir
from concourse import bass_utils

F32 = mybir.dt.float32
F16 = mybir.dt.float16
ActF = mybir.ActivationFunctionType

B, L, C, H, Dh = 4, 1024, 1280, 20, 64
NCORES = 8
HPC = 10               # heads per core
NP = HPC // 2          # head pairs per core
ALPHA = 0.8 * 0.6
LB = 256               # bank keys per head after 2x2 pooling
KEYS = L + LB          # 1280
KCH = KEYS // 128      # 10 key chunks
CC = C // 128          # 10 contraction chunks
LT = L // 128          # 8 query/row tiles


def _build():
    nc = bacc.Bacc("TRN2", target_bir_lowering=False, debug=False,
                   num_devices=NCORES)
    hsT_d = nc.dram_tensor("hsT", [C, L], F16, kind="ExternalInput")
    # wq/wk pre-arranged on host as [NP][128 part][CC][128 cols]
    wq_d = nc.dram_tensor("wq", [NP, 128, CC, 128], F16, kind="ExternalInput")
    wk_d = nc.dram_tensor("wk", [NP, 128, CC, 128], F16, kind="ExternalInput")
    # wv pre-arranged as [2 halves][128 part][CC][320 cols]
    wv_d = nc.dram_tensor("wv", [2, 128, CC, 320], F16, kind="ExternalInput")
    wo_d = nc.dram_tensor("wo", [HPC * Dh, C], F16, kind="ExternalInput")
    kbT_d = nc.dram_tensor("kbT", [HPC * Dh, LB], F16, kind="ExternalInput")
    vb_d = nc.dram_tensor("vb", [LB, HPC * Dh], F16, kind="ExternalInput")
    out_d = nc.dram_tensor("out", [L, C], F16, kind="ExternalOutput")

    with tile.TileContext(nc) as tc, ExitStack() as es:
        big = es.enter_context(tc.tile_pool(name="big", bufs=1))
        wst = es.enter_context(tc.tile_pool(name="wst", bufs=2))
        qkt = es.enter_context(tc.tile_pool(name="qkt", bufs=2))
        expp = es.enter_context(tc.tile_pool(name="expp", bufs=3))
        rcp = es.enter_context(tc.tile_pool(name="rcp", bufs=2))
        wop = es.enter_context(tc.tile_pool(name="wop", bufs=1))
        outp = es.enter_context(tc.tile_pool(name="outp", bufs=3))

        ctxT_sb = big.tile([128, NP, L], F16)
        v_sb = big.tile([128, KCH, HPC * 128], F16)
        v_heads = v_sb[:].rearrange("p c (h x) -> p c h x", x=128)
        hsT_sb = big.tile([128, CC, L], F16)

        # ---- startup DMAs: each queue leads with a cc=0-critical tile ----
        nc.sync.dma_start(hsT_sb[:, 0, :], hsT_d.ap()[0:128, :])
        wq0 = wst.tile([128, CC, 128], F16, tag="wq", name="wq0")
        nc.sync.dma_start(wq0[:], wq_d.ap()[0])
        wk0 = wst.tile([128, CC, 128], F16, tag="wk", name="wk0")
        nc.scalar.dma_start(wk0[:], wk_d.ap()[0])
        nc.scalar.dma_start(hsT_sb[:, 1, :], hsT_d.ap()[128:256, :])
        wv0 = wst.tile([128, CC, 320], F16, tag="wv", name="wv0", bufs=1)
        nc.gpsimd.dma_start(wv0[:], wv_d.ap()[0])
        # hsT chunks 2..9 round-robin over the three queues
        qs = [nc.gpsimd, nc.sync, nc.scalar]
        for cc in range(2, CC):
            qs[cc % 3].dma_start(hsT_sb[:, cc, :],
                                 hsT_d.ap()[cc * 128:(cc + 1) * 128, :])
        wv1 = wst.tile([128, CC, 320], F16, tag="wv", name="wv1", bufs=1)
        nc.scalar.dma_start(wv1[:], wv_d.ap()[1])
        vbt = big.tile([128, 2, HPC, Dh], F16)
        for j in range(LB // 128):
            nc.gpsimd.dma_start(
                vbt[:, j], vb_d.ap()[j * 128:(j + 1) * 128, :]
                .rearrange("p (h d) -> p h d", d=Dh))

        qts, kts = {}, {}
        qts[0] = qkt.tile([128, L], F16, tag="qT", name="qT0")
        kts[0] = qkt.tile([128, KEYS], F16, tag="kT", name="kT0")
        nc.sync.dma_start(kts[0][:, L:KEYS], kbT_d.ap()[0:128, :])

        # ---- startup compute: cc-outer proj pair0 + vproj g0 lt0-3 ----
        st_es = ExitStack()
        stp = st_es.enter_context(tc.tile_pool(name="stp", bufs=1,
                                               space="PSUM"))
        pq = [stp.tile([128, 512], F32, tag=f"pq{h}", name=f"pq{h}")
              for h in range(2)]
        pk = [stp.tile([128, 512], F32, tag=f"pk{h}", name=f"pk{h}")
              for h in range(2)]
        pv = [stp.tile([128, 320], F32, tag=f"pv{lt}", name=f"spv{lt}")
              for lt in range(4)]
        # q/k loop first: it only needs hsT chunks + the small wq0/wk0, so
        # the in-order PE stream never stalls on the larger wv0 transfer
        for cc in range(CC):
            st = (cc == 0)
            sp = (cc == CC - 1)
            for h in range(2):
                nc.tensor.matmul(pq[h][:], wq0[:, cc, :],
                                 hsT_sb[:, cc, h * 512:(h + 1) * 512],
                                 start=st, stop=sp)
            for h in range(2):
                nc.tensor.matmul(pk[h][:], wk0[:, cc, :],
                                 hsT_sb[:, cc, h * 512:(h + 1) * 512],
                                 start=st, stop=sp)
        for cc in range(CC):
            for lt in range(4):
                nc.tensor.matmul(pv[lt][:],
                                 hsT_sb[:, cc, lt * 128:(lt + 1) * 128],
                                 wv0[:, cc, :],
                                 start=(cc == 0), stop=(cc == CC - 1))
        # h0 halves first (unblock scores kc0-3), ACT+DVE in parallel
        nc.scalar.activation(kts[0][:, 0:512], pk[0][:], ActF.Copy)
        nc.vector.tensor_copy(qts[0][:, 0:512], pq[0][:])
        nc.scalar.activation(kts[0][:, 512:1024], pk[1][:], ActF.Copy)
        nc.vector.tensor_copy(qts[0][:, 512:1024], pq[1][:])
        for lt in range(4):
            nc.vector.tensor_copy(
                v_heads[:, lt, 0:5, 0:Dh],
                pv[lt][:].rearrange("p (h d) -> p h d", d=Dh))
        st_es.close()

        # ones columns (denominator trick) + bank V columns
        ones32 = big.tile([128, HPC, Dh], F16)
        nc.vector.memset(ones32[:], 1.0)
        for kc in range(KCH):
            nc.vector.tensor_copy(v_heads[:, kc, :, Dh:128], ones32[:])
        for j in range(LB // 128):
            nc.vector.tensor_copy(v_heads[:, LT + j, :, 0:Dh], vbt[:, j])

        # ---- attention-phase PSUM pools (banks freed by st_es) ----
        at_es = ExitStack()
        pss = at_es.enter_context(tc.tile_pool(name="pss", bufs=2,
                                               space="PSUM"))
        psc = at_es.enter_context(tc.tile_pool(name="psc", bufs=2,
                                               space="PSUM"))
        psj = at_es.enter_context(tc.tile_pool(name="psj", bufs=2,
                                               space="PSUM"))

        # ---- interleavable filler emitters ----
        def emit_vproj_tile(g, lt):
            wv_sb = wv0 if g == 0 else wv1
            pvt = psj.tile([128, 512], F32, tag="pj", name=f"pv{g}_{lt}")
            for cc in range(CC):
                nc.tensor.matmul(pvt[:, 0:320],
                                 hsT_sb[:, cc, lt * 128:(lt + 1) * 128],
                                 wv_sb[:, cc, :],
                                 start=(cc == 0), stop=(cc == CC - 1))
            nc.vector.tensor_copy(
                v_heads[:, lt, g * 5:(g + 1) * 5, 0:Dh],
                pvt[:, 0:320].rearrange("p (h d) -> p h d", d=Dh))

        def emit_proj_half(m, which, h):
            """One query-half of the q or k projection for pair m."""
            if which == "q":
                if h == 0:
                    qts[m] = qkt.tile([128, L], F16, tag="qT", name=f"qT{m}")
                dst, w_d, wtag = qts[m], wq_d, "wq"
            else:
                if h == 0:
                    kts[m] = qkt.tile([128, KEYS], F16, tag="kT",
                                      name=f"kT{m}")
                dst, w_d, wtag = kts[m], wk_d, "wk"
            if h == 0:
                w_sb = wst.tile([128, CC, 128], F16, tag=wtag,
                                name=f"{wtag}{m}")
                nc.sync.dma_start(w_sb[:], w_d.ap()[m])
                if which == "k":
                    nc.sync.dma_start(dst[:, L:KEYS],
                                      kbT_d.ap()[m * 128:(m + 1) * 128, :])
                proj_w[(m, wtag)] = w_sb
            w_sb = proj_w[(m, wtag)]
            pp = psj.tile([128, 512], F32, tag="pj", name=f"p{wtag}{m}_{h}")
            for cc in range(CC):
                nc.tensor.matmul(pp[:], w_sb[:, cc, :],
                                 hsT_sb[:, cc, h * 512:(h + 1) * 512],
                                 start=(cc == 0), stop=(cc == CC - 1))
            nc.vector.tensor_copy(dst[:, h * 512:(h + 1) * 512], pp[:])

        proj_w = {(0, "wq"): wq0, (0, "wk"): wk0}

        wo_tiles = []

        def emit_wo_dma(p):
            wo_sb = wop.tile([128, C], F16, tag=f"wo{p}", name=f"wo{p}")
            nc.sync.dma_start(wo_sb[:], wo_d.ap()[p * 128:(p + 1) * 128, :])
            wo_tiles.append(wo_sb)

        def emit_outproj(qt, n0, nsz, early=False):
            if early:
                po = psj.tile([128, 512], F32, tag="pj", name=f"po{qt}_{n0}")
            else:
                po = pss.tile([128, 1024], F32, tag="ss", name=f"po{qt}_{n0}")
            for p in range(NP):
                nc.tensor.matmul(
                    po[:, 0:nsz],
                    ctxT_sb[:, p, qt * 128:(qt + 1) * 128],
                    wo_tiles[p][:, n0:n0 + nsz],
                    start=(p == 0), stop=(p == NP - 1))
            ob = outp.tile([128, 512], F16, tag="ob", name=f"ob{qt}_{n0}")
            # alternate PSUM evacuation between DVE and the post-exp-idle ACT
            if (qt * 3 + n0 // 512) % 2 == 0:
                nc.vector.tensor_copy(ob[:, 0:nsz], po[:, 0:nsz])
            else:
                nc.scalar.activation(ob[:, 0:nsz], po[:, 0:nsz], ActF.Copy)
            (nc.sync if (qt + n0 // 512) % 2 == 0 else nc.gpsimd).dma_start(
                out_d.ap()[qt * 128:(qt + 1) * 128, n0:n0 + nsz],
                ob[:, 0:nsz])

        # filler schedule: (m, half, kc) -> list of thunks
        filler = {}

        def add_filler(m, half, kc, fn):
            filler.setdefault((m, half, kc), []).append(fn)

        # vproj g0 lt4-7: two pre-loop (cover the startup-evac window), two in
        # p0h0; g1 spread over p0h1/p1h0. Projections split q-in-h0 / k-in-h1.
        add_filler(0, 0, -1, lambda: emit_vproj_tile(0, 4))
        add_filler(0, 0, -1, lambda: emit_vproj_tile(0, 5))
        add_filler(0, 0, 4, lambda: emit_vproj_tile(0, 6))
        add_filler(0, 0, 6, lambda: emit_vproj_tile(0, 7))
        add_filler(0, 1, 4, lambda: emit_vproj_tile(1, 0))
        add_filler(0, 1, 8, lambda: emit_vproj_tile(1, 1))
        add_filler(1, 0, 2, lambda: emit_vproj_tile(1, 2))
        add_filler(1, 0, 6, lambda: emit_vproj_tile(1, 3))
        add_filler(1, 1, 4, lambda: emit_vproj_tile(1, 4))
        add_filler(1, 1, 8, lambda: emit_vproj_tile(1, 5))
        add_filler(2, 0, 1, lambda: emit_vproj_tile(1, 6))
        add_filler(2, 0, 5, lambda: emit_vproj_tile(1, 7))
        for m in range(NP - 1):
            add_filler(m, 0, 2, lambda m=m: emit_proj_half(m + 1, "q", 0))
            add_filler(m, 0, 8, lambda m=m: emit_proj_half(m + 1, "q", 1))
            add_filler(m, 1, 2, lambda m=m: emit_proj_half(m + 1, "k", 0))
            add_filler(m, 1, 6, lambda m=m: emit_proj_half(m + 1, "k", 1))
        # wo preload during pair 1
        for p in range(NP):
            add_filler(1, 0, 2 * p + 1, lambda p=p: emit_wo_dma(p))
        # out-proj for query tiles 0-3 inside pair 4 half 1
        for i, (qt, n0, nsz) in enumerate(
                [(qt, n0, nsz) for qt in range(4)
                 for n0, nsz in ((0, 512), (512, 512), (1024, 256))]):
            add_filler(4, 1, i % 10,
                       lambda qt=qt, n0=n0, nsz=nsz: emit_outproj(
                           qt, n0, nsz, early=True))

        # ---- attention main loop ----
        for m in range(NP):
            for half in range(2):
                ctxps = [psc.tile([128, 512], F32, tag="ctx",
                                  name=f"ctx{m}_{half}_{par}")
                         for par in range(2)]
                for fn in filler.get((m, half, -1), ()):
                    fn()
                es_ = {}
                for kc in range(KCH):
                    ss = pss.tile([128, 1024], F32, tag="ss",
                                  name=f"s{m}_{half}_{kc}")
                    for par in range(2):
                        p0 = 64 * par
                        nc.tensor.matmul(
                            ss[:, par * 512:(par + 1) * 512],
                            kts[m][p0:p0 + 64, kc * 128:(kc + 1) * 128],
                            qts[m][p0:p0 + 64,
                                   half * 512:(half + 1) * 512],
                            start=True, stop=True, tile_position=(p0, 0))
                    e = expp.tile([128, 1024], F16, tag="e",
                                  name=f"e{m}_{half}_{kc}")
                    nc.scalar.activation(e[:], ss[:], ActF.Exp, scale=0.125)
                    es_[kc] = e
                    if kc >= 1:
                        ep = es_.pop(kc - 1)
                        for par in range(2):
                            nc.tensor.matmul(
                                ctxps[par][:],
                                v_heads[:, kc - 1, 2 * m + par, :],
                                ep[:, par * 512:(par + 1) * 512],
                                start=(kc - 1 == 0), stop=False)
                    for fn in filler.get((m, half, kc), ()):
                        fn()
                ep = es_.pop(KCH - 1)
                for par in range(2):
                    nc.tensor.matmul(
                        ctxps[par][:],
                        v_heads[:, KCH - 1, 2 * m + par, :],
                        ep[:, par * 512:(par + 1) * 512],
                        start=False, stop=True)
                # normalize: copy denominator rows to SBUF, approx-recip, mul
                if m < NP - 1:
                    for par in range(2):
                        dn = rcp.tile([64, 512], F32, tag="dn",
                                      name=f"dn{m}_{half}_{par}")
                        nc.vector.tensor_copy(dn[:], ctxps[par][64:128, :])
                        rc = rcp.tile([64, 512], F32, tag="rc",
                                      name=f"rc{m}_{half}_{par}")
                        nc.vector.reciprocal_approx_fast(rc[:], dn[:])
                        nc.vector.tensor_mul(
                            ctxT_sb[64 * par:64 * par + 64, m,
                                    half * 512:(half + 1) * 512],
                            ctxps[par][0:64, :], rc[:])
                else:
                    # last pair gates the out-projection: dn copies on the
                    # drained ACT, muls chunked per query tile so each
                    # out-proj qt unblocks as soon as its 128 cols are done
                    rcs = []
                    for par in range(2):
                        dn = rcp.tile([64, 512], F32, tag="dn",
                                      name=f"dn{m}_{half}_{par}")
                        nc.scalar.activation(dn[:], ctxps[par][64:128, :],
                                             ActF.Copy)
                        rc = rcp.tile([64, 512], F32, tag="rc",
                                      name=f"rc{m}_{half}_{par}")
                        nc.vector.reciprocal_approx_fast(rc[:], dn[:])
                        rcs.append(rc)
                    for i in range(4):
                        cs = slice(i * 128, (i + 1) * 128)
                        for par in range(2):
                            nc.vector.tensor_mul(
                                ctxT_sb[64 * par:64 * par + 64, m,
                                        half * 512 + i * 128:
                                        half * 512 + (i + 1) * 128],
                                ctxps[par][0:64, cs], rcs[par][:, cs])

        # ---- output projection, query tiles 4-7 (0-3 emitted above) ----
        for qt in range(4, LT):
            for n0, nsz in ((0, 512), (512, 512), (1024, 256)):
                emit_outproj(qt, n0, nsz)
        at_es.close()
    nc.compile()
    return nc


_NC = None


def _get_nc():
    global _NC
    if _NC is None:
        _NC = _build()
    return _NC


def _prep_in_maps(hidden_states, Wq, Wk, Wv, Wo, K_bg, V_bg):
    hs = np.asarray(hidden_states, np.float32)
    Wq, Wk, Wv, Wo = (np.asarray(w, np.float32) for w in (Wq, Wk, Wv, Wo))
    K_bg = np.asarray(K_bg, np.float32)
    V_bg = np.asarray(V_bg, np.float32)

    hsT = [np.ascontiguousarray(hs[bi].T).astype(np.float16)
           for bi in range(B)]

    def lay_qk(w, g):  # [1280, 640] slice -> [NP, 128, CC, 128]
        sl = w[:, g * 640:(g + 1) * 640]           # [C, 640]
        a = sl.reshape(CC, 128, NP, 128)           # (cc, p, m, n)
        return np.ascontiguousarray(a.transpose(2, 1, 0, 3)).astype(np.float16)

    def lay_wv(w, g):  # [1280, 640] slice -> [2, 128, CC, 320]
        sl = w[:, g * 640:(g + 1) * 640]
        a = sl.reshape(CC, 128, 2, 320)            # (cc, p, gg, n)
        return np.ascontiguousarray(a.transpose(2, 1, 0, 3)).astype(np.float16)

    wq_s = [lay_qk(Wq, g) for g in range(2)]
    wk_s = [lay_qk(Wk, g) for g in range(2)]
    wv_s = [lay_wv(Wv, g) for g in range(2)]
    wo_s = [Wo[g * 640:(g + 1) * 640, :].astype(np.float16) for g in range(2)]

    def pool_bank(x):  # [10, 1024, 64] -> [10, 256, 64], fp16 round + alpha
        x = x.astype(np.float16).astype(np.float32)
        x = x.reshape(HPC, 16, 2, 16, 2, Dh).mean(axis=(2, 4))
        return (ALPHA * x).reshape(HPC, LB, Dh)

    kb_s, vb_s = [], []
    for base in (0, 10, 20, 30):
        kb = pool_bank(K_bg[base:base + HPC])
        vb = pool_bank(V_bg[base:base + HPC])
        kb_s.append(kb.transpose(0, 2, 1).reshape(HPC * Dh, LB).astype(np.float16))
        vb_s.append(vb.transpose(1, 0, 2).reshape(LB, HPC * Dh).astype(np.float16))

    in_maps = []
    for c in range(NCORES):
        bi, g = c // 2, c % 2
        bank = (20 * bi + 10 * g) % 40 // 10
        in_maps.append({
            "hsT": hsT[bi], "wq": wq_s[g], "wk": wk_s[g], "wv": wv_s[g],
            "wo": wo_s[g], "kbT": kb_s[bank], "vb": vb_s[bank],
        })
    return in_maps


def _run(in_maps, **kwargs):
    return bass_utils.run_bass_kernel_spmd(
        _get_nc(), in_maps, core_ids=list(range(NCORES)), **kwargs)


def kernel(hidden_states, Wq, Wk, Wv, Wo, bo, K_bg, V_bg):
    in_maps = _prep_in_maps(hidden_states, Wq, Wk, Wv, Wo, K_bg, V_bg)
    res = _run(in_maps)
    bo = np.asarray(bo, np.float32)
    out = np.empty((B, L, C), np.float32)
    for bi in range(B):
        out[bi] = (res.results[2 * bi]["out"].astype(np.float32)
                   + res.results[2 * bi + 1]["out"].astype(np.float32)
                   + bo[None, :])
    return out


# revision 18
# speedup vs baseline: 1.0009x; 1.0009x over previous
"""CARC attention processor kernel for 8 Trainium2 NeuronCores.

Sharding: data-parallel over the fused B*H axis. 80 heads / 8 cores =
10 heads per core; each core owns one batch (bi = core//2) and one
10-head group (g = core%2). Projection weights are column/row-sliced
per head group; the KV bank is sliced per core. Each core emits a
partial output projection over its 640 channels (fp16); the host sums
the two partials per batch and adds the bias.

Device schedule per core (all matmuls fp16 in, fp32 PSUM):
  - startup, cc-outer over chunk-wise multi-queue hsT DMA: q/k
    projections for pair 0 plus vproj lt0-3 start as soon as the first
    128-row hsT chunk lands (8 PSUM banks: 2+2 proj halves + 4 vproj).
  - attention per pair, per query-half (512 cols): scores are emitted
    as adjacent K=64 matmul pairs at tile_position (0,0)/(64,0) so the
    two heads stream the PE concurrently; one [128,1024] exp (ACT)
    covers both heads; ctx accumulates into 1-bank [128,512] tiles
    (64 ctx rows + 64 ones-denominator rows).
  - remaining vproj tiles and the next pair's q/k projections are
    interleaved into the exp-gated slack of the kc loop.
  - normalization: reciprocal_approx_fast on the denominator rows read
    straight from PSUM, then one tensor_mul into ctxT (fp16).
  - out-projection for query tiles 0-3 overlaps the last pair's second
    half; wo preloads on the idle SP queue during pair 1.
"""
from contextlib import ExitStack

import numpy as np

import concourse.bass as bass
import concourse.tile as tile
from concourse import bacc, mybir
from concourse import bass_utils

F32 = mybir.dt.float32
F16 = mybir.dt.float16
ActF = mybir.ActivationFunctionType

B, L, C, H, Dh = 4, 1024, 1280, 20, 64
NCORES = 8
HPC = 10               # heads per core
NP = HPC // 2          # head pairs per core
ALPHA = 0.8 * 0.6
LB = 256               # bank keys per head after 2x2 pooling
KEYS = L + LB          # 1280
KCH = KEYS // 128      # 10 key chunks
CC = C // 128          # 10 contraction chunks
LT = L // 128          # 8 query/row tiles


def _build():
    nc = bacc.Bacc("TRN2", target_bir_lowering=False, debug=False,
                   num_devices=NCORES)
    hsT_d = nc.dram_tensor("hsT", [C, L], F16, kind="ExternalInput")
    # wq/wk pre-arranged on host as [NP][128 part][CC][128 cols]
    wq_d = nc.dram_tensor("wq", [NP, 128, CC, 128], F16, kind="ExternalInput")
    wk_d = nc.dram_tensor("wk", [NP, 128, CC, 128], F16, kind="ExternalInput")
    # wv pre-arranged as [2 halves][128 part][CC][320 cols]
    wv_d = nc.dram_tensor("wv", [2, 128, CC, 320], F16, kind="ExternalInput")
    wo_d = nc.dram_tensor("wo", [HPC * Dh, C], F16, kind="ExternalInput")
    kbT_d = nc.dram_tensor("kbT", [HPC * Dh, LB], F16, kind="ExternalInput")
    vb_d = nc.dram_tensor("vb", [LB, HPC * Dh], F16, kind="ExternalInput")
    out_d = nc.dram_tensor("out", [L, C], F16, kind="ExternalOutput")

    with tile.TileContext(nc) as tc, ExitStack() as es:
        big = es.enter_context(tc.tile_pool(name="big", bufs=1))
        wst = es.enter_context(tc.tile_pool(name="wst", bufs=2))
        qkt = es.enter_context(tc.tile_pool(name="qkt", bufs=2))
        expp = es.enter_context(tc.tile_pool(name="expp", bufs=3))
        rcp = es.enter_context(tc.tile_pool(name="rcp", bufs=2))
        wop = es.enter_context(tc.tile_pool(name="wop", bufs=1))
        outp = es.enter_context(tc.tile_pool(name="outp", bufs=3))

        ctxT_sb = big.tile([128, NP, L], F16)
        v_sb = big.tile([128, KCH, HPC * 128], F16)
        v_heads = v_sb[:].rearrange("p c (h x) -> p c h x", x=128)
        hsT_sb = big.tile([128, CC, L], F16)

        # ---- startup DMAs: each queue leads with a cc=0-critical tile ----
        nc.sync.dma_start(hsT_sb[:, 0, :], hsT_d.ap()[0:128, :])
        wq0 = wst.tile([128, CC, 128], F16, tag="wq", name="wq0")
        nc.sync.dma_start(wq0[:], wq_d.ap()[0])
        wk0 = wst.tile([128, CC, 128], F16, tag="wk", name="wk0")
        nc.scalar.dma_start(wk0[:], wk_d.ap()[0])
        wv0 = wst.tile([128, CC, 320], F16, tag="wv", name="wv0", bufs=1)
        nc.gpsimd.dma_start(wv0[:], wv_d.ap()[0])
        # hsT chunks: arrival order matched to the cc-loop consumption order
        for q, cc in ((nc.scalar, 1), (nc.sync, 2), (nc.gpsimd, 3),
                      (nc.scalar, 4), (nc.sync, 5), (nc.gpsimd, 6),
                      (nc.scalar, 7), (nc.sync, 8), (nc.gpsimd, 9)):
            q.dma_start(hsT_sb[:, cc, :],
                        hsT_d.ap()[cc * 128:(cc + 1) * 128, :])
        wv1 = wst.tile([128, CC, 320], F16, tag="wv", name="wv1", bufs=1)
        nc.scalar.dma_start(wv1[:], wv_d.ap()[1])
        vbt = big.tile([128, 2, HPC, Dh], F16)
        for j in range(LB // 128):
            nc.gpsimd.dma_start(
                vbt[:, j], vb_d.ap()[j * 128:(j + 1) * 128, :]
                .rearrange("p (h d) -> p h d", d=Dh))

        qts, kts = {}, {}
        qts[0] = qkt.tile([128, L], F16, tag="qT", name="qT0")
        kts[0] = qkt.tile([128, KEYS], F16, tag="kT", name="kT0")
        nc.sync.dma_start(kts[0][:, L:KEYS], kbT_d.ap()[0:128, :])

        # ---- startup compute: cc-outer proj pair0 + vproj g0 lt0-3 ----
        st_es = ExitStack()
        stp = st_es.enter_context(tc.tile_pool(name="stp", bufs=1,
                                               space="PSUM"))
        pq = [stp.tile([128, 512], F32, tag=f"pq{h}", name=f"pq{h}")
              for h in range(2)]
        pk = [stp.tile([128, 512], F32, tag=f"pk{h}", name=f"pk{h}")
              for h in range(2)]
        pv = [stp.tile([128, 320], F32, tag=f"pv{lt}", name=f"spv{lt}")
              for lt in range(4)]
        # q/k lead; vproj lags 2 chunks so the in-order PE stream consumes
        # each hsT chunk in ~1.7us (>= DMA arrival cadence) and never
        # stalls on the larger wv0 transfer
        for cc in range(CC + 2):
            if cc < CC:
                st = (cc == 0)
                sp = (cc == CC - 1)
                for h in range(2):
                    nc.tensor.matmul(pq[h][:], wq0[:, cc, :],
                                     hsT_sb[:, cc, h * 512:(h + 1) * 512],
                                     start=st, stop=sp)
                for h in range(2):
                    nc.tensor.matmul(pk[h][:], wk0[:, cc, :],
                                     hsT_sb[:, cc, h * 512:(h + 1) * 512],
                                     start=st, stop=sp)
            if cc >= 2:
                for lt in range(4):
                    nc.tensor.matmul(pv[lt][:],
                                     hsT_sb[:, cc - 2, lt * 128:(lt + 1) * 128],
                                     wv0[:, cc - 2, :],
                                     start=(cc == 2), stop=(cc == CC + 1))
        # h0 halves first (unblock scores kc0-3), ACT+DVE in parallel
        nc.scalar.activation(kts[0][:, 0:512], pk[0][:], ActF.Copy)
        nc.vector.tensor_copy(qts[0][:, 0:512], pq[0][:])
        nc.scalar.activation(kts[0][:, 512:1024], pk[1][:], ActF.Copy)
        nc.vector.tensor_copy(qts[0][:, 512:1024], pq[1][:])
        for lt in range(4):
            nc.vector.tensor_copy(
                v_heads[:, lt, 0:5, 0:Dh],
                pv[lt][:].rearrange("p (h d) -> p h d", d=Dh))
        st_es.close()

        # ones columns (denominator trick) + bank V columns
        ones32 = big.tile([128, HPC, Dh], F16)
        nc.vector.memset(ones32[:], 1.0)
        for kc in range(KCH):
            nc.vector.tensor_copy(v_heads[:, kc, :, Dh:128], ones32[:])
        for j in range(LB // 128):
            nc.vector.tensor_copy(v_heads[:, LT + j, :, 0:Dh], vbt[:, j])

        # ---- attention-phase PSUM pools (banks freed by st_es) ----
        at_es = ExitStack()
        pss = at_es.enter_context(tc.tile_pool(name="pss", bufs=2,
                                               space="PSUM"))
        psc = at_es.enter_context(tc.tile_pool(name="psc", bufs=2,
                                               space="PSUM"))
        psj = at_es.enter_context(tc.tile_pool(name="psj", bufs=2,
                                               space="PSUM"))

        # ---- interleavable filler emitters ----
        def emit_vproj_tile(g, lt):
            wv_sb = wv0 if g == 0 else wv1
            pvt = psj.tile([128, 512], F32, tag="pj", name=f"pv{g}_{lt}")
            for cc in range(CC):
                nc.tensor.matmul(pvt[:, 0:320],
                                 hsT_sb[:, cc, lt * 128:(lt + 1) * 128],
                                 wv_sb[:, cc, :],
                                 start=(cc == 0), stop=(cc == CC - 1))
            nc.vector.tensor_copy(
                v_heads[:, lt, g * 5:(g + 1) * 5, 0:Dh],
                pvt[:, 0:320].rearrange("p (h d) -> p h d", d=Dh))

        def emit_proj_half(m, which, h):
            """One query-half of the q or k projection for pair m."""
            if which == "q":
                if h == 0:
                    qts[m] = qkt.tile([128, L], F16, tag="qT", name=f"qT{m}")
                dst, w_d, wtag = qts[m], wq_d, "wq"
            else:
                if h == 0:
                    kts[m] = qkt.tile([128, KEYS], F16, tag="kT",
                                      name=f"kT{m}")
                dst, w_d, wtag = kts[m], wk_d, "wk"
            if h == 0:
                w_sb = wst.tile([128, CC, 128], F16, tag=wtag,
                                name=f"{wtag}{m}")
                nc.sync.dma_start(w_sb[:], w_d.ap()[m])
                if which == "k":
                    nc.sync.dma_start(dst[:, L:KEYS],
                                      kbT_d.ap()[m * 128:(m + 1) * 128, :])
                proj_w[(m, wtag)] = w_sb
            w_sb = proj_w[(m, wtag)]
            pp = psj.tile([128, 512], F32, tag="pj", name=f"p{wtag}{m}_{h}")
            for cc in range(CC):
                nc.tensor.matmul(pp[:], w_sb[:, cc, :],
                                 hsT_sb[:, cc, h * 512:(h + 1) * 512],
                                 start=(cc == 0), stop=(cc == CC - 1))
            nc.vector.tensor_copy(dst[:, h * 512:(h + 1) * 512], pp[:])

        proj_w = {(0, "wq"): wq0, (0, "wk"): wk0}

        wo_tiles = []

        def emit_wo_dma(p):
            wo_sb = wop.tile([128, C], F16, tag=f"wo{p}", name=f"wo{p}")
            nc.sync.dma_start(wo_sb[:], wo_d.ap()[p * 128:(p + 1) * 128, :])
            wo_tiles.append(wo_sb)

        def emit_outproj(qt, n0, nsz, early=False):
            if early:
                po = psj.tile([128, 512], F32, tag="pj", name=f"po{qt}_{n0}")
            else:
                po = pss.tile([128, 1024], F32, tag="ss", name=f"po{qt}_{n0}")
            for p in range(NP):
                nc.tensor.matmul(
                    po[:, 0:nsz],
                    ctxT_sb[:, p, qt * 128:(qt + 1) * 128],
                    wo_tiles[p][:, n0:n0 + nsz],
                    start=(p == 0), stop=(p == NP - 1))
            ob = outp.tile([128, 512], F16, tag="ob", name=f"ob{qt}_{n0}")
            # alternate PSUM evacuation between DVE and the post-exp-idle ACT
            if (qt * 3 + n0 // 512) % 2 == 0:
                nc.vector.tensor_copy(ob[:, 0:nsz], po[:, 0:nsz])
            else:
                nc.scalar.activation(ob[:, 0:nsz], po[:, 0:nsz], ActF.Copy)
            (nc.sync if (qt + n0 // 512) % 2 == 0 else nc.gpsimd).dma_start(
                out_d.ap()[qt * 128:(qt + 1) * 128, n0:n0 + nsz],
                ob[:, 0:nsz])

        # filler schedule: (m, half, kc) -> list of thunks
        filler = {}

        def add_filler(m, half, kc, fn):
            filler.setdefault((m, half, kc), []).append(fn)

        # vproj g0 lt4-7: two pre-loop (cover the startup-evac window), two in
        # p0h0; g1 spread over p0h1/p1h0. Projections split q-in-h0 / k-in-h1.
        add_filler(0, 0, -1, lambda: emit_vproj_tile(0, 4))
        add_filler(0, 0, -1, lambda: emit_vproj_tile(0, 5))
        add_filler(0, 0, 4, lambda: emit_vproj_tile(0, 6))
        add_filler(0, 0, 6, lambda: emit_vproj_tile(0, 7))
        add_filler(0, 1, 4, lambda: emit_vproj_tile(1, 0))
        add_filler(0, 1, 8, lambda: emit_vproj_tile(1, 1))
        add_filler(1, 0, 2, lambda: emit_vproj_tile(1, 2))
        add_filler(1, 0, 6, lambda: emit_vproj_tile(1, 3))
        add_filler(1, 1, 4, lambda: emit_vproj_tile(1, 4))
        add_filler(1, 1, 8, lambda: emit_vproj_tile(1, 5))
        add_filler(2, 0, 1, lambda: emit_vproj_tile(1, 6))
        add_filler(2, 0, 5, lambda: emit_vproj_tile(1, 7))
        for m in range(NP - 1):
            add_filler(m, 0, 2, lambda m=m: emit_proj_half(m + 1, "q", 0))
            add_filler(m, 0, 8, lambda m=m: emit_proj_half(m + 1, "q", 1))
            add_filler(m, 1, 2, lambda m=m: emit_proj_half(m + 1, "k", 0))
            add_filler(m, 1, 6, lambda m=m: emit_proj_half(m + 1, "k", 1))
        # wo preload during pair 1
        for p in range(NP):
            add_filler(1, 0, 2 * p + 1, lambda p=p: emit_wo_dma(p))
        # out-proj for query tiles 0-3 inside pair 4 half 1
        for i, (qt, n0, nsz) in enumerate(
                [(qt, n0, nsz) for qt in range(4)
                 for n0, nsz in ((0, 512), (512, 512), (1024, 256))]):
            add_filler(4, 1, i % 10,
                       lambda qt=qt, n0=n0, nsz=nsz: emit_outproj(
                           qt, n0, nsz, early=True))

        # ---- attention main loop ----
        for m in range(NP):
            for half in range(2):
                ctxps = [psc.tile([128, 512], F32, tag="ctx",
                                  name=f"ctx{m}_{half}_{par}")
                         for par in range(2)]
                for fn in filler.get((m, half, -1), ()):
                    fn()
                es_ = {}
                for kc in range(KCH):
                    ss = pss.tile([128, 1024], F32, tag="ss",
                                  name=f"s{m}_{half}_{kc}")
                    for par in range(2):
                        p0 = 64 * par
                        nc.tensor.matmul(
                            ss[:, par * 512:(par + 1) * 512],
                            kts[m][p0:p0 + 64, kc * 128:(kc + 1) * 128],
                            qts[m][p0:p0 + 64,
                                   half * 512:(half + 1) * 512],
                            start=True, stop=True, tile_position=(p0, 0))
                    e = expp.tile([128, 1024], F16, tag="e",
                                  name=f"e{m}_{half}_{kc}")
                    nc.scalar.activation(e[:], ss[:], ActF.Exp, scale=0.125)
                    es_[kc] = e
                    if kc >= 1:
                        ep = es_.pop(kc - 1)
                        for par in range(2):
                            nc.tensor.matmul(
                                ctxps[par][:],
                                v_heads[:, kc - 1, 2 * m + par, :],
                                ep[:, par * 512:(par + 1) * 512],
                                start=(kc - 1 == 0), stop=False)
                    for fn in filler.get((m, half, kc), ()):
                        fn()
                ep = es_.pop(KCH - 1)
                for par in range(2):
                    nc.tensor.matmul(
                        ctxps[par][:],
                        v_heads[:, KCH - 1, 2 * m + par, :],
                        ep[:, par * 512:(par + 1) * 512],
                        start=False, stop=True)
                # normalize: copy denominator rows to SBUF, approx-recip, mul
                if m < NP - 1:
                    for par in range(2):
                        dn = rcp.tile([64, 512], F32, tag="dn",
                                      name=f"dn{m}_{half}_{par}")
                        nc.vector.tensor_copy(dn[:], ctxps[par][64:128, :])
                        rc = rcp.tile([64, 512], F32, tag="rc",
                                      name=f"rc{m}_{half}_{par}")
                        nc.vector.reciprocal_approx_fast(rc[:], dn[:])
                        nc.vector.tensor_mul(
                            ctxT_sb[64 * par:64 * par + 64, m,
                                    half * 512:(half + 1) * 512],
                            ctxps[par][0:64, :], rc[:])
                else:
                    # last pair gates the out-projection: dn copies on the
                    # drained ACT, muls chunked per query tile so each
                    # out-proj qt unblocks as soon as its 128 cols are done
                    rcs = []
                    for par in range(2):
                        dn = rcp.tile([64, 512], F32, tag="dn",
                                      name=f"dn{m}_{half}_{par}")
                        nc.scalar.activation(dn[:], ctxps[par][64:128, :],
                                             ActF.Copy)
                        rc = rcp.tile([64, 512], F32, tag="rc",
                                      name=f"rc{m}_{half}_{par}")
                        nc.vector.reciprocal_approx_fast(rc[:], dn[:])
                        rcs.append(rc)
                    for i in range(4):
                        cs = slice(i * 128, (i + 1) * 128)
                        for par in range(2):
                            nc.vector.tensor_mul(
                                ctxT_sb[64 * par:64 * par + 64, m,
                                        half * 512 + i * 128:
                                        half * 512 + (i + 1) * 128],
                                ctxps[par][0:64, cs], rcs[par][:, cs])

        # ---- output projection, query tiles 4-7 (0-3 emitted above) ----
        for qt in range(4, LT):
            for n0, nsz in ((0, 512), (512, 512), (1024, 256)):
                emit_outproj(qt, n0, nsz)
        at_es.close()
    nc.compile()
    return nc


_NC = None


def _get_nc():
    global _NC
    if _NC is None:
        _NC = _build()
    return _NC


def _prep_in_maps(hidden_states, Wq, Wk, Wv, Wo, K_bg, V_bg):
    hs = np.asarray(hidden_states, np.float32)
    Wq, Wk, Wv, Wo = (np.asarray(w, np.float32) for w in (Wq, Wk, Wv, Wo))
    K_bg = np.asarray(K_bg, np.float32)
    V_bg = np.asarray(V_bg, np.float32)

    hsT = [np.ascontiguousarray(hs[bi].T).astype(np.float16)
           for bi in range(B)]

    def lay_qk(w, g):  # [1280, 640] slice -> [NP, 128, CC, 128]
        sl = w[:, g * 640:(g + 1) * 640]           # [C, 640]
        a = sl.reshape(CC, 128, NP, 128)           # (cc, p, m, n)
        return np.ascontiguousarray(a.transpose(2, 1, 0, 3)).astype(np.float16)

    def lay_wv(w, g):  # [1280, 640] slice -> [2, 128, CC, 320]
        sl = w[:, g * 640:(g + 1) * 640]
        a = sl.reshape(CC, 128, 2, 320)            # (cc, p, gg, n)
        return np.ascontiguousarray(a.transpose(2, 1, 0, 3)).astype(np.float16)

    wq_s = [lay_qk(Wq, g) for g in range(2)]
    wk_s = [lay_qk(Wk, g) for g in range(2)]
    wv_s = [lay_wv(Wv, g) for g in range(2)]
    wo_s = [Wo[g * 640:(g + 1) * 640, :].astype(np.float16) for g in range(2)]

    def pool_bank(x):  # [10, 1024, 64] -> [10, 256, 64], fp16 round + alpha
        x = x.astype(np.float16).astype(np.float32)
        x = x.reshape(HPC, 16, 2, 16, 2, Dh).mean(axis=(2, 4))
        return (ALPHA * x).reshape(HPC, LB, Dh)

    kb_s, vb_s = [], []
    for base in (0, 10, 20, 30):
        kb = pool_bank(K_bg[base:base + HPC])
        vb = pool_bank(V_bg[base:base + HPC])
        kb_s.append(kb.transpose(0, 2, 1).reshape(HPC * Dh, LB).astype(np.float16))
        vb_s.append(vb.transpose(1, 0, 2).reshape(LB, HPC * Dh).astype(np.float16))

    in_maps = []
    for c in range(NCORES):
        bi, g = c // 2, c % 2
        bank = (20 * bi + 10 * g) % 40 // 10
        in_maps.append({
            "hsT": hsT[bi], "wq": wq_s[g], "wk": wk_s[g], "wv": wv_s[g],
            "wo": wo_s[g], "kbT": kb_s[bank], "vb": vb_s[bank],
        })
    return in_maps


def _run(in_maps, **kwargs):
    return bass_utils.run_bass_kernel_spmd(
        _get_nc(), in_maps, core_ids=list(range(NCORES)), **kwargs)


def kernel(hidden_states, Wq, Wk, Wv, Wo, bo, K_bg, V_bg):
    in_maps = _prep_in_maps(hidden_states, Wq, Wk, Wv, Wo, K_bg, V_bg)
    res = _run(in_maps)
    bo = np.asarray(bo, np.float32)
    out = np.empty((B, L, C), np.float32)
    for bi in range(B):
        out[bi] = (res.results[2 * bi]["out"].astype(np.float32)
                   + res.results[2 * bi + 1]["out"].astype(np.float32)
                   + bo[None, :])
    return out


# revision 19
# speedup vs baseline: 1.0088x; 1.0079x over previous
"""CARC attention processor kernel for 8 Trainium2 NeuronCores.

Sharding: data-parallel over the fused B*H axis. 80 heads / 8 cores =
10 heads per core; each core owns one batch (bi = core//2) and one
10-head group (g = core%2). Projection weights are column/row-sliced
per head group; the KV bank is sliced per core. Each core emits a
partial output projection over its 640 channels (fp16); the host sums
the two partials per batch and adds the bias.

Device schedule per core (all matmuls fp16 in, fp32 PSUM):
  - startup, cc-outer over chunk-wise multi-queue hsT DMA: q/k
    projections for pair 0 plus vproj lt0-3 start as soon as the first
    128-row hsT chunk lands (8 PSUM banks: 2+2 proj halves + 4 vproj).
  - attention per pair, per query-half (512 cols): scores are emitted
    as adjacent K=64 matmul pairs at tile_position (0,0)/(64,0) so the
    two heads stream the PE concurrently; one [128,1024] exp (ACT)
    covers both heads; ctx accumulates into 1-bank [128,512] tiles
    (64 ctx rows + 64 ones-denominator rows).
  - remaining vproj tiles and the next pair's q/k projections are
    interleaved into the exp-gated slack of the kc loop.
  - normalization: reciprocal_approx_fast on the denominator rows read
    straight from PSUM, then one tensor_mul into ctxT (fp16).
  - out-projection for query tiles 0-3 overlaps the last pair's second
    half; wo preloads on the idle SP queue during pair 1.
"""
from contextlib import ExitStack

import numpy as np

import concourse.bass as bass
import concourse.tile as tile
from concourse import bacc, mybir
from concourse import bass_utils

F32 = mybir.dt.float32
F16 = mybir.dt.float16
ActF = mybir.ActivationFunctionType

B, L, C, H, Dh = 4, 1024, 1280, 20, 64
NCORES = 8
HPC = 10               # heads per core
NP = HPC // 2          # head pairs per core
ALPHA = 0.8 * 0.6
LB = 256               # bank keys per head after 2x2 pooling
KEYS = L + LB          # 1280
KCH = KEYS // 128      # 10 key chunks
CC = C // 128          # 10 contraction chunks
LT = L // 128          # 8 query/row tiles


def _build():
    nc = bacc.Bacc("TRN2", target_bir_lowering=False, debug=False,
                   num_devices=NCORES)
    hsT_d = nc.dram_tensor("hsT", [C, L], F16, kind="ExternalInput")
    # wq/wk pre-arranged on host as [NP][128 part][CC][128 cols]
    wq_d = nc.dram_tensor("wq", [NP, 128, CC, 128], F16, kind="ExternalInput")
    wk_d = nc.dram_tensor("wk", [NP, 128, CC, 128], F16, kind="ExternalInput")
    # wv pre-arranged as [2 halves][128 part][CC][320 cols]
    wv_d = nc.dram_tensor("wv", [2, 128, CC, 320], F16, kind="ExternalInput")
    wo_d = nc.dram_tensor("wo", [HPC * Dh, C], F16, kind="ExternalInput")
    kbT_d = nc.dram_tensor("kbT", [HPC * Dh, LB], F16, kind="ExternalInput")
    vb_d = nc.dram_tensor("vb", [LB, HPC * Dh], F16, kind="ExternalInput")
    out_d = nc.dram_tensor("out", [L, C], F16, kind="ExternalOutput")

    with tile.TileContext(nc) as tc, ExitStack() as es:
        big = es.enter_context(tc.tile_pool(name="big", bufs=1))
        wst = es.enter_context(tc.tile_pool(name="wst", bufs=2))
        qkt = es.enter_context(tc.tile_pool(name="qkt", bufs=2))
        expp = es.enter_context(tc.tile_pool(name="expp", bufs=3))
        rcp = es.enter_context(tc.tile_pool(name="rcp", bufs=2))
        wop = es.enter_context(tc.tile_pool(name="wop", bufs=1))
        outp = es.enter_context(tc.tile_pool(name="outp", bufs=3))

        ctxT_sb = big.tile([128, NP, L], F16)
        v_sb = big.tile([128, KCH, HPC * 128], F16)
        v_heads = v_sb[:].rearrange("p c (h x) -> p c h x", x=128)
        hsT_sb = big.tile([128, CC, L], F16)

        # ---- startup DMAs: each queue leads with a cc=0-critical tile ----
        nc.sync.dma_start(hsT_sb[:, 0, :], hsT_d.ap()[0:128, :])
        wq0 = wst.tile([128, CC, 128], F16, tag="wq", name="wq0")
        nc.sync.dma_start(wq0[:], wq_d.ap()[0])
        wk0 = wst.tile([128, CC, 128], F16, tag="wk", name="wk0")
        nc.gpsimd.dma_start(wk0[:], wk_d.ap()[0])
        wv0 = wst.tile([128, CC, 320], F16, tag="wv", name="wv0", bufs=1)
        nc.gpsimd.dma_start(wv0[:], wv_d.ap()[0])
        # hsT chunks: arrival order matched to the cc-loop consumption order
        for q, cc in ((nc.scalar, 1), (nc.sync, 2), (nc.scalar, 3),
                      (nc.gpsimd, 4), (nc.sync, 5), (nc.scalar, 6),
                      (nc.gpsimd, 7), (nc.sync, 8), (nc.scalar, 9)):
            q.dma_start(hsT_sb[:, cc, :],
                        hsT_d.ap()[cc * 128:(cc + 1) * 128, :])
        wv1 = wst.tile([128, CC, 320], F16, tag="wv", name="wv1", bufs=1)
        nc.scalar.dma_start(wv1[:], wv_d.ap()[1])
        vbt = big.tile([128, 2, HPC, Dh], F16)
        for j in range(LB // 128):
            nc.gpsimd.dma_start(
                vbt[:, j], vb_d.ap()[j * 128:(j + 1) * 128, :]
                .rearrange("p (h d) -> p h d", d=Dh))

        qts, kts = {}, {}
        qts[0] = qkt.tile([128, L], F16, tag="qT", name="qT0")
        kts[0] = qkt.tile([128, KEYS], F16, tag="kT", name="kT0")
        nc.sync.dma_start(kts[0][:, L:KEYS], kbT_d.ap()[0:128, :])

        # ---- startup compute: cc-outer proj pair0 + vproj g0 lt0-3 ----
        st_es = ExitStack()
        stp = st_es.enter_context(tc.tile_pool(name="stp", bufs=1,
                                               space="PSUM"))
        pq = [stp.tile([128, 512], F32, tag=f"pq{h}", name=f"pq{h}")
              for h in range(2)]
        pk = [stp.tile([128, 512], F32, tag=f"pk{h}", name=f"pk{h}")
              for h in range(2)]
        pv = [stp.tile([128, 320], F32, tag=f"pv{lt}", name=f"spv{lt}")
              for lt in range(4)]
        # q/k lead; vproj lags 2 chunks so the in-order PE stream consumes
        # each hsT chunk in ~1.7us (>= DMA arrival cadence) and never
        # stalls on the larger wv0 transfer
        for cc in range(CC + 2):
            if cc < CC:
                st = (cc == 0)
                sp = (cc == CC - 1)
                for h in range(2):
                    nc.tensor.matmul(pq[h][:], wq0[:, cc, :],
                                     hsT_sb[:, cc, h * 512:(h + 1) * 512],
                                     start=st, stop=sp)
                for h in range(2):
                    nc.tensor.matmul(pk[h][:], wk0[:, cc, :],
                                     hsT_sb[:, cc, h * 512:(h + 1) * 512],
                                     start=st, stop=sp)
            if cc >= 2:
                for lt in range(4):
                    nc.tensor.matmul(pv[lt][:],
                                     hsT_sb[:, cc - 2, lt * 128:(lt + 1) * 128],
                                     wv0[:, cc - 2, :],
                                     start=(cc == 2), stop=(cc == CC + 1))
        # h0 halves first (unblock scores kc0-3), ACT+DVE in parallel
        nc.scalar.activation(kts[0][:, 0:512], pk[0][:], ActF.Copy)
        nc.vector.tensor_copy(qts[0][:, 0:512], pq[0][:])
        nc.scalar.activation(kts[0][:, 512:1024], pk[1][:], ActF.Copy)
        nc.vector.tensor_copy(qts[0][:, 512:1024], pq[1][:])
        for lt in range(4):
            nc.vector.tensor_copy(
                v_heads[:, lt, 0:5, 0:Dh],
                pv[lt][:].rearrange("p (h d) -> p h d", d=Dh))
        st_es.close()

        # ones columns (denominator trick) + bank V columns
        ones32 = big.tile([128, HPC, Dh], F16)
        nc.vector.memset(ones32[:], 1.0)
        for kc in range(KCH):
            nc.vector.tensor_copy(v_heads[:, kc, :, Dh:128], ones32[:])
        for j in range(LB // 128):
            nc.vector.tensor_copy(v_heads[:, LT + j, :, 0:Dh], vbt[:, j])

        # ---- attention-phase PSUM pools (banks freed by st_es) ----
        at_es = ExitStack()
        pss = at_es.enter_context(tc.tile_pool(name="pss", bufs=2,
                                               space="PSUM"))
        psc = at_es.enter_context(tc.tile_pool(name="psc", bufs=2,
                                               space="PSUM"))
        psj = at_es.enter_context(tc.tile_pool(name="psj", bufs=2,
                                               space="PSUM"))

        # ---- interleavable filler emitters ----
        def emit_vproj_tile(g, lt):
            wv_sb = wv0 if g == 0 else wv1
            pvt = psj.tile([128, 512], F32, tag="pj", name=f"pv{g}_{lt}")
            for cc in range(CC):
                nc.tensor.matmul(pvt[:, 0:320],
                                 hsT_sb[:, cc, lt * 128:(lt + 1) * 128],
                                 wv_sb[:, cc, :],
                                 start=(cc == 0), stop=(cc == CC - 1))
            nc.vector.tensor_copy(
                v_heads[:, lt, g * 5:(g + 1) * 5, 0:Dh],
                pvt[:, 0:320].rearrange("p (h d) -> p h d", d=Dh))

        def emit_proj_half(m, which, h):
            """One query-half of the q or k projection for pair m."""
            if which == "q":
                if h == 0:
                    qts[m] = qkt.tile([128, L], F16, tag="qT", name=f"qT{m}")
                dst, w_d, wtag = qts[m], wq_d, "wq"
            else:
                if h == 0:
                    kts[m] = qkt.tile([128, KEYS], F16, tag="kT",
                                      name=f"kT{m}")
                dst, w_d, wtag = kts[m], wk_d, "wk"
            if h == 0:
                w_sb = wst.tile([128, CC, 128], F16, tag=wtag,
                                name=f"{wtag}{m}")
                nc.sync.dma_start(w_sb[:], w_d.ap()[m])
                if which == "k":
                    nc.sync.dma_start(dst[:, L:KEYS],
                                      kbT_d.ap()[m * 128:(m + 1) * 128, :])
                proj_w[(m, wtag)] = w_sb
            w_sb = proj_w[(m, wtag)]
            pp = psj.tile([128, 512], F32, tag="pj", name=f"p{wtag}{m}_{h}")
            for cc in range(CC):
                nc.tensor.matmul(pp[:], w_sb[:, cc, :],
                                 hsT_sb[:, cc, h * 512:(h + 1) * 512],
                                 start=(cc == 0), stop=(cc == CC - 1))
            nc.vector.tensor_copy(dst[:, h * 512:(h + 1) * 512], pp[:])

        proj_w = {(0, "wq"): wq0, (0, "wk"): wk0}

        wo_tiles = []

        def emit_wo_dma(p):
            wo_sb = wop.tile([128, C], F16, tag=f"wo{p}", name=f"wo{p}")
            nc.sync.dma_start(wo_sb[:], wo_d.ap()[p * 128:(p + 1) * 128, :])
            wo_tiles.append(wo_sb)

        def emit_outproj(qt, n0, nsz, early=False):
            if early:
                po = psj.tile([128, 512], F32, tag="pj", name=f"po{qt}_{n0}")
            else:
                po = pss.tile([128, 1024], F32, tag="ss", name=f"po{qt}_{n0}")
            for p in range(NP):
                nc.tensor.matmul(
                    po[:, 0:nsz],
                    ctxT_sb[:, p, qt * 128:(qt + 1) * 128],
                    wo_tiles[p][:, n0:n0 + nsz],
                    start=(p == 0), stop=(p == NP - 1))
            ob = outp.tile([128, 512], F16, tag="ob", name=f"ob{qt}_{n0}")
            # alternate PSUM evacuation between DVE and the post-exp-idle ACT
            if (qt * 3 + n0 // 512) % 2 == 0:
                nc.vector.tensor_copy(ob[:, 0:nsz], po[:, 0:nsz])
            else:
                nc.scalar.activation(ob[:, 0:nsz], po[:, 0:nsz], ActF.Copy)
            (nc.sync if (qt + n0 // 512) % 2 == 0 else nc.gpsimd).dma_start(
                out_d.ap()[qt * 128:(qt + 1) * 128, n0:n0 + nsz],
                ob[:, 0:nsz])

        # filler schedule: (m, half, kc) -> list of thunks
        filler = {}

        def add_filler(m, half, kc, fn):
            filler.setdefault((m, half, kc), []).append(fn)

        # vproj g0 lt4-7: two pre-loop (cover the startup-evac window), two in
        # p0h0; g1 spread over p0h1/p1h0. Projections split q-in-h0 / k-in-h1.
        add_filler(0, 0, -1, lambda: emit_vproj_tile(0, 4))
        add_filler(0, 0, -1, lambda: emit_vproj_tile(0, 5))
        add_filler(0, 0, 4, lambda: emit_vproj_tile(0, 6))
        add_filler(0, 0, 6, lambda: emit_vproj_tile(0, 7))
        add_filler(0, 1, 4, lambda: emit_vproj_tile(1, 0))
        add_filler(0, 1, 8, lambda: emit_vproj_tile(1, 1))
        add_filler(1, 0, 2, lambda: emit_vproj_tile(1, 2))
        add_filler(1, 0, 6, lambda: emit_vproj_tile(1, 3))
        add_filler(1, 1, 4, lambda: emit_vproj_tile(1, 4))
        add_filler(1, 1, 8, lambda: emit_vproj_tile(1, 5))
        add_filler(2, 0, 1, lambda: emit_vproj_tile(1, 6))
        add_filler(2, 0, 5, lambda: emit_vproj_tile(1, 7))
        for m in range(NP - 1):
            add_filler(m, 0, 2, lambda m=m: emit_proj_half(m + 1, "q", 0))
            add_filler(m, 0, 8, lambda m=m: emit_proj_half(m + 1, "q", 1))
            add_filler(m, 1, 2, lambda m=m: emit_proj_half(m + 1, "k", 0))
            add_filler(m, 1, 6, lambda m=m: emit_proj_half(m + 1, "k", 1))
        # wo preload during pair 1
        for p in range(NP):
            add_filler(1, 0, 2 * p + 1, lambda p=p: emit_wo_dma(p))
        # out-proj for query tiles 0-3 inside pair 4 half 1
        for i, (qt, n0, nsz) in enumerate(
                [(qt, n0, nsz) for qt in range(4)
                 for n0, nsz in ((0, 512), (512, 512), (1024, 256))]):
            add_filler(4, 1, i % 10,
                       lambda qt=qt, n0=n0, nsz=nsz: emit_outproj(
                           qt, n0, nsz, early=True))

        # ---- attention main loop ----
        for m in range(NP):
            for half in range(2):
                ctxps = [psc.tile([128, 512], F32, tag="ctx",
                                  name=f"ctx{m}_{half}_{par}")
                         for par in range(2)]
                for fn in filler.get((m, half, -1), ()):
                    fn()
                es_ = {}
                for kc in range(KCH):
                    ss = pss.tile([128, 1024], F32, tag="ss",
                                  name=f"s{m}_{half}_{kc}")
                    for par in range(2):
                        p0 = 64 * par
                        nc.tensor.matmul(
                            ss[:, par * 512:(par + 1) * 512],
                            kts[m][p0:p0 + 64, kc * 128:(kc + 1) * 128],
                            qts[m][p0:p0 + 64,
                                   half * 512:(half + 1) * 512],
                            start=True, stop=True, tile_position=(p0, 0))
                    e = expp.tile([128, 1024], F16, tag="e",
                                  name=f"e{m}_{half}_{kc}")
                    nc.scalar.activation(e[:], ss[:], ActF.Exp, scale=0.125)
                    es_[kc] = e
                    if kc >= 1:
                        ep = es_.pop(kc - 1)
                        for par in range(2):
                            nc.tensor.matmul(
                                ctxps[par][:],
                                v_heads[:, kc - 1, 2 * m + par, :],
                                ep[:, par * 512:(par + 1) * 512],
                                start=(kc - 1 == 0), stop=False)
                    for fn in filler.get((m, half, kc), ()):
                        fn()
                ep = es_.pop(KCH - 1)
                for par in range(2):
                    nc.tensor.matmul(
                        ctxps[par][:],
                        v_heads[:, KCH - 1, 2 * m + par, :],
                        ep[:, par * 512:(par + 1) * 512],
                        start=False, stop=True)
                # normalize: copy denominator rows to SBUF, approx-recip, mul
                if m < NP - 1:
                    for par in range(2):
                        dn = rcp.tile([64, 512], F32, tag="dn",
                                      name=f"dn{m}_{half}_{par}")
                        nc.vector.tensor_copy(dn[:], ctxps[par][64:128, :])
                        rc = rcp.tile([64, 512], F32, tag="rc",
                                      name=f"rc{m}_{half}_{par}")
                        nc.vector.reciprocal_approx_fast(rc[:], dn[:])
                        nc.vector.tensor_mul(
                            ctxT_sb[64 * par:64 * par + 64, m,
                                    half * 512:(half + 1) * 512],
                            ctxps[par][0:64, :], rc[:])
                else:
                    # last pair gates the out-projection: dn copies on the
                    # drained ACT, muls chunked per query tile so each
                    # out-proj qt unblocks as soon as its 128 cols are done
                    rcs = []
                    for par in range(2):
                        dn = rcp.tile([64, 512], F32, tag="dn",
                                      name=f"dn{m}_{half}_{par}")
                        nc.scalar.activation(dn[:], ctxps[par][64:128, :],
                                             ActF.Copy)
                        rc = rcp.tile([64, 512], F32, tag="rc",
                                      name=f"rc{m}_{half}_{par}")
                        nc.vector.reciprocal_approx_fast(rc[:], dn[:])
                        rcs.append(rc)
                    for i in range(4):
                        cs = slice(i * 128, (i + 1) * 128)
                        for par in range(2):
                            nc.vector.tensor_mul(
                                ctxT_sb[64 * par:64 * par + 64, m,
                                        half * 512 + i * 128:
                                        half * 512 + (i + 1) * 128],
                                ctxps[par][0:64, cs], rcs[par][:, cs])

        # ---- output projection, query tiles 4-7 (0-3 emitted above) ----
        for qt in range(4, LT):
            for n0, nsz in ((0, 512), (512, 512), (1024, 256)):
                emit_outproj(qt, n0, nsz)
        at_es.close()
    nc.compile()
    return nc


_NC = None


def _get_nc():
    global _NC
    if _NC is None:
        _NC = _build()
    return _NC


def _prep_in_maps(hidden_states, Wq, Wk, Wv, Wo, K_bg, V_bg):
    hs = np.asarray(hidden_states, np.float32)
    Wq, Wk, Wv, Wo = (np.asarray(w, np.float32) for w in (Wq, Wk, Wv, Wo))
    K_bg = np.asarray(K_bg, np.float32)
    V_bg = np.asarray(V_bg, np.float32)

    hsT = [np.ascontiguousarray(hs[bi].T).astype(np.float16)
           for bi in range(B)]

    def lay_qk(w, g):  # [1280, 640] slice -> [NP, 128, CC, 128]
        sl = w[:, g * 640:(g + 1) * 640]           # [C, 640]
        a = sl.reshape(CC, 128, NP, 128)           # (cc, p, m, n)
        return np.ascontiguousarray(a.transpose(2, 1, 0, 3)).astype(np.float16)

    def lay_wv(w, g):  # [1280, 640] slice -> [2, 128, CC, 320]
        sl = w[:, g * 640:(g + 1) * 640]
        a = sl.reshape(CC, 128, 2, 320)            # (cc, p, gg, n)
        return np.ascontiguousarray(a.transpose(2, 1, 0, 3)).astype(np.float16)

    wq_s = [lay_qk(Wq, g) for g in range(2)]
    wk_s = [lay_qk(Wk, g) for g in range(2)]
    wv_s = [lay_wv(Wv, g) for g in range(2)]
    wo_s = [Wo[g * 640:(g + 1) * 640, :].astype(np.float16) for g in range(2)]

    def pool_bank(x):  # [10, 1024, 64] -> [10, 256, 64], fp16 round + alpha
        x = x.astype(np.float16).astype(np.float32)
        x = x.reshape(HPC, 16, 2, 16, 2, Dh).mean(axis=(2, 4))
        return (ALPHA * x).reshape(HPC, LB, Dh)

    kb_s, vb_s = [], []
    for base in (0, 10, 20, 30):
        kb = pool_bank(K_bg[base:base + HPC])
        vb = pool_bank(V_bg[base:base + HPC])
        kb_s.append(kb.transpose(0, 2, 1).reshape(HPC * Dh, LB).astype(np.float16))
        vb_s.append(vb.transpose(1, 0, 2).reshape(LB, HPC * Dh).astype(np.float16))

    in_maps = []
    for c in range(NCORES):
        bi, g = c // 2, c % 2
        bank = (20 * bi + 10 * g) % 40 // 10
        in_maps.append({
            "hsT": hsT[bi], "wq": wq_s[g], "wk": wk_s[g], "wv": wv_s[g],
            "wo": wo_s[g], "kbT": kb_s[bank], "vb": vb_s[bank],
        })
    return in_maps


def _run(in_maps, **kwargs):
    return bass_utils.run_bass_kernel_spmd(
        _get_nc(), in_maps, core_ids=list(range(NCORES)), **kwargs)


def kernel(hidden_states, Wq, Wk, Wv, Wo, bo, K_bg, V_bg):
    in_maps = _prep_in_maps(hidden_states, Wq, Wk, Wv, Wo, K_bg, V_bg)
    res = _run(in_maps)
    bo = np.asarray(bo, np.float32)
    out = np.empty((B, L, C), np.float32)
    for bi in range(B):
        out[bi] = (res.results[2 * bi]["out"].astype(np.float32)
                   + res.results[2 * bi + 1]["out"].astype(np.float32)
                   + bo[None, :])
    return out


# revision 20
# speedup vs baseline: 1.0189x; 1.0100x over previous
"""CARC attention processor kernel for 8 Trainium2 NeuronCores.

Sharding: data-parallel over the fused B*H axis. 80 heads / 8 cores =
10 heads per core; each core owns one batch (bi = core//2) and one
10-head group (g = core%2). Projection weights are column/row-sliced
per head group; the KV bank is sliced per core. Each core emits a
partial output projection over its 640 channels (fp16); the host sums
the two partials per batch and adds the bias.

Device schedule per core (all matmuls fp16 in, fp32 PSUM):
  - startup, cc-outer over chunk-wise multi-queue hsT DMA: q/k
    projections for pair 0 plus vproj lt0-3 start as soon as the first
    128-row hsT chunk lands (8 PSUM banks: 2+2 proj halves + 4 vproj).
  - attention per pair, per query-half (512 cols): scores are emitted
    as adjacent K=64 matmul pairs at tile_position (0,0)/(64,0) so the
    two heads stream the PE concurrently; one [128,1024] exp (ACT)
    covers both heads; ctx accumulates into 1-bank [128,512] tiles
    (64 ctx rows + 64 ones-denominator rows).
  - remaining vproj tiles and the next pair's q/k projections are
    interleaved into the exp-gated slack of the kc loop.
  - normalization: reciprocal_approx_fast on the denominator rows read
    straight from PSUM, then one tensor_mul into ctxT (fp16).
  - out-projection for query tiles 0-3 overlaps the last pair's second
    half; wo preloads on the idle SP queue during pair 1.
"""
from contextlib import ExitStack

import numpy as np

import concourse.bass as bass
import concourse.tile as tile
from concourse import bacc, mybir
from concourse import bass_utils

F32 = mybir.dt.float32
F16 = mybir.dt.float16
ActF = mybir.ActivationFunctionType

B, L, C, H, Dh = 4, 1024, 1280, 20, 64
NCORES = 8
HPC = 10               # heads per core
NP = HPC // 2          # head pairs per core
ALPHA = 0.8 * 0.6
LB = 256               # bank keys per head after 2x2 pooling
KEYS = L + LB          # 1280
KCH = KEYS // 128      # 10 key chunks
CC = C // 128          # 10 contraction chunks
LT = L // 128          # 8 query/row tiles


def _build():
    nc = bacc.Bacc("TRN2", target_bir_lowering=False, debug=False,
                   num_devices=NCORES)
    hsT_d = nc.dram_tensor("hsT", [C, L], F16, kind="ExternalInput")
    # wq/wk pre-arranged on host as [NP][128 part][CC][128 cols]
    wq_d = nc.dram_tensor("wq", [NP, 128, CC, 128], F16, kind="ExternalInput")
    wk_d = nc.dram_tensor("wk", [NP, 128, CC, 128], F16, kind="ExternalInput")
    # wv pre-arranged as [2 halves][128 part][CC][320 cols]
    wv_d = nc.dram_tensor("wv", [2, 128, CC, 320], F16, kind="ExternalInput")
    wo_d = nc.dram_tensor("wo", [HPC * Dh, C], F16, kind="ExternalInput")
    kbT_d = nc.dram_tensor("kbT", [HPC * Dh, LB], F16, kind="ExternalInput")
    vb_d = nc.dram_tensor("vb", [LB, HPC * Dh], F16, kind="ExternalInput")
    out_d = nc.dram_tensor("out", [L, C], F16, kind="ExternalOutput")

    with tile.TileContext(nc) as tc, ExitStack() as es:
        big = es.enter_context(tc.tile_pool(name="big", bufs=1))
        wst = es.enter_context(tc.tile_pool(name="wst", bufs=2))
        qkt = es.enter_context(tc.tile_pool(name="qkt", bufs=2))
        expp = es.enter_context(tc.tile_pool(name="expp", bufs=3))
        rcp = es.enter_context(tc.tile_pool(name="rcp", bufs=2))
        wop = es.enter_context(tc.tile_pool(name="wop", bufs=1))
        outp = es.enter_context(tc.tile_pool(name="outp", bufs=3))

        ctxT_sb = big.tile([128, NP, L], F16)
        v_sb = big.tile([128, KCH, HPC * 128], F16)
        v_heads = v_sb[:].rearrange("p c (h x) -> p c h x", x=128)
        hsT_sb = big.tile([128, CC, L], F16)

        # ---- startup DMAs: each queue leads with a cc=0-critical tile ----
        nc.sync.dma_start(hsT_sb[:, 0, :], hsT_d.ap()[0:128, :])
        wq0 = wst.tile([128, CC, 128], F16, tag="wq", name="wq0")
        nc.sync.dma_start(wq0[:], wq_d.ap()[0])
        wk0 = wst.tile([128, CC, 128], F16, tag="wk", name="wk0")
        nc.gpsimd.dma_start(wk0[:], wk_d.ap()[0])
        wv0 = wst.tile([128, CC, 320], F16, tag="wv", name="wv0", bufs=1)
        nc.gpsimd.dma_start(wv0[:], wv_d.ap()[0])
        # hsT chunks: arrival order matched to the cc-loop consumption order
        for q, cc in ((nc.scalar, 1), (nc.sync, 2), (nc.scalar, 3),
                      (nc.gpsimd, 4), (nc.sync, 5), (nc.scalar, 6),
                      (nc.gpsimd, 7), (nc.sync, 8), (nc.scalar, 9)):
            q.dma_start(hsT_sb[:, cc, :],
                        hsT_d.ap()[cc * 128:(cc + 1) * 128, :])
        wv1 = wst.tile([128, CC, 320], F16, tag="wv", name="wv1", bufs=1)
        nc.scalar.dma_start(wv1[:], wv_d.ap()[1])
        vbt = big.tile([128, 2, HPC, Dh], F16)
        for j in range(LB // 128):
            nc.gpsimd.dma_start(
                vbt[:, j], vb_d.ap()[j * 128:(j + 1) * 128, :]
                .rearrange("p (h d) -> p h d", d=Dh))

        qts, kts = {}, {}
        qts[0] = qkt.tile([128, L], F16, tag="qT", name="qT0")
        kts[0] = qkt.tile([128, KEYS], F16, tag="kT", name="kT0")
        nc.sync.dma_start(kts[0][:, L:KEYS], kbT_d.ap()[0:128, :])

        # ---- startup compute: cc-outer proj pair0 + vproj g0 lt0-3 ----
        st_es = ExitStack()
        stp = st_es.enter_context(tc.tile_pool(name="stp", bufs=1,
                                               space="PSUM"))
        pq = [stp.tile([128, 512], F32, tag=f"pq{h}", name=f"pq{h}")
              for h in range(2)]
        pk = [stp.tile([128, 512], F32, tag=f"pk{h}", name=f"pk{h}")
              for h in range(2)]
        pv = [stp.tile([128, 320], F32, tag=f"pv{lt}", name=f"spv{lt}")
              for lt in range(4)]
        # q/k lead; vproj lags 2 chunks so the in-order PE stream consumes
        # each hsT chunk in ~1.7us (>= DMA arrival cadence) and never
        # stalls on the larger wv0 transfer
        for cc in range(CC + 2):
            if cc < CC:
                st = (cc == 0)
                sp = (cc == CC - 1)
                for h in range(2):
                    nc.tensor.matmul(pq[h][:], wq0[:, cc, :],
                                     hsT_sb[:, cc, h * 512:(h + 1) * 512],
                                     start=st, stop=sp)
                for h in range(2):
                    nc.tensor.matmul(pk[h][:], wk0[:, cc, :],
                                     hsT_sb[:, cc, h * 512:(h + 1) * 512],
                                     start=st, stop=sp)
            if cc >= 2:
                for lt in range(4):
                    nc.tensor.matmul(pv[lt][:],
                                     hsT_sb[:, cc - 2, lt * 128:(lt + 1) * 128],
                                     wv0[:, cc - 2, :],
                                     start=(cc == 2), stop=(cc == CC + 1))
        # h0 halves first (unblock scores kc0-3), ACT+DVE in parallel
        nc.scalar.activation(kts[0][:, 0:512], pk[0][:], ActF.Copy)
        nc.vector.tensor_copy(qts[0][:, 0:512], pq[0][:])
        nc.scalar.activation(kts[0][:, 512:1024], pk[1][:], ActF.Copy)
        nc.vector.tensor_copy(qts[0][:, 512:1024], pq[1][:])
        for lt in range(4):
            nc.vector.tensor_copy(
                v_heads[:, lt, 0:5, 0:Dh],
                pv[lt][:].rearrange("p (h d) -> p h d", d=Dh))
        st_es.close()

        # ones columns (denominator trick) + bank V columns
        ones32 = big.tile([128, HPC, Dh], F16)
        nc.vector.memset(ones32[:], 1.0)
        for kc in range(KCH):
            nc.vector.tensor_copy(v_heads[:, kc, :, Dh:128], ones32[:])
        for j in range(LB // 128):
            nc.vector.tensor_copy(v_heads[:, LT + j, :, 0:Dh], vbt[:, j])

        # ---- attention-phase PSUM pools (banks freed by st_es) ----
        at_es = ExitStack()
        pss = at_es.enter_context(tc.tile_pool(name="pss", bufs=2,
                                               space="PSUM"))
        psc = at_es.enter_context(tc.tile_pool(name="psc", bufs=2,
                                               space="PSUM"))
        psj = at_es.enter_context(tc.tile_pool(name="psj", bufs=2,
                                               space="PSUM"))

        # ---- interleavable filler emitters ----
        def emit_vproj_tile(g, lt):
            wv_sb = wv0 if g == 0 else wv1
            pvt = psj.tile([128, 512], F32, tag="pj", name=f"pv{g}_{lt}")
            for cc in range(CC):
                nc.tensor.matmul(pvt[:, 0:320],
                                 hsT_sb[:, cc, lt * 128:(lt + 1) * 128],
                                 wv_sb[:, cc, :],
                                 start=(cc == 0), stop=(cc == CC - 1))
            nc.vector.tensor_copy(
                v_heads[:, lt, g * 5:(g + 1) * 5, 0:Dh],
                pvt[:, 0:320].rearrange("p (h d) -> p h d", d=Dh))

        def emit_proj_half(m, which, h):
            """One query-half of the q or k projection for pair m."""
            if which == "q":
                if h == 0:
                    qts[m] = qkt.tile([128, L], F16, tag="qT", name=f"qT{m}")
                dst, w_d, wtag = qts[m], wq_d, "wq"
            else:
                if h == 0:
                    kts[m] = qkt.tile([128, KEYS], F16, tag="kT",
                                      name=f"kT{m}")
                dst, w_d, wtag = kts[m], wk_d, "wk"
            if h == 0:
                w_sb = wst.tile([128, CC, 128], F16, tag=wtag,
                                name=f"{wtag}{m}")
                nc.sync.dma_start(w_sb[:], w_d.ap()[m])
                if which == "k":
                    nc.sync.dma_start(dst[:, L:KEYS],
                                      kbT_d.ap()[m * 128:(m + 1) * 128, :])
                proj_w[(m, wtag)] = w_sb
            w_sb = proj_w[(m, wtag)]
            pp = psj.tile([128, 512], F32, tag="pj", name=f"p{wtag}{m}_{h}")
            for cc in range(CC):
                nc.tensor.matmul(pp[:], w_sb[:, cc, :],
                                 hsT_sb[:, cc, h * 512:(h + 1) * 512],
                                 start=(cc == 0), stop=(cc == CC - 1))
            nc.vector.tensor_copy(dst[:, h * 512:(h + 1) * 512], pp[:])

        proj_w = {(0, "wq"): wq0, (0, "wk"): wk0}

        wo_tiles = []

        def emit_wo_dma(p):
            wo_sb = wop.tile([128, C], F16, tag=f"wo{p}", name=f"wo{p}")
            nc.sync.dma_start(wo_sb[:], wo_d.ap()[p * 128:(p + 1) * 128, :])
            wo_tiles.append(wo_sb)

        op_cnt = [0]

        def emit_outproj(qt, n0, nsz, early=False):
            # late chunks alternate pss/psj slots (4-deep po rotation: both
            # pools are free once scores and fillers have drained)
            use_pj = early or (not early and op_cnt[0] % 2 == 1)
            if use_pj:
                po = psj.tile([128, 512], F32, tag="pj", name=f"po{qt}_{n0}")
            else:
                po = pss.tile([128, 1024], F32, tag="ss", name=f"po{qt}_{n0}")
            for p in range(NP):
                nc.tensor.matmul(
                    po[:, 0:nsz],
                    ctxT_sb[:, p, qt * 128:(qt + 1) * 128],
                    wo_tiles[p][:, n0:n0 + nsz],
                    start=(p == 0), stop=(p == NP - 1))
            ob = outp.tile([128, 512], F16, tag="ob", name=f"ob{qt}_{n0}")
            # early chunks alternate DVE/ACT; late chunks go all-ACT so DVE
            # is free for the final normalization muls that gate the LDWs
            if early and (qt * 3 + n0 // 512) % 2 == 0:
                nc.vector.tensor_copy(ob[:, 0:nsz], po[:, 0:nsz])
            else:
                nc.scalar.activation(ob[:, 0:nsz], po[:, 0:nsz], ActF.Copy)
            op_cnt[0] += 1
            (nc.sync if (qt + n0 // 512) % 2 == 0 else nc.gpsimd).dma_start(
                out_d.ap()[qt * 128:(qt + 1) * 128, n0:n0 + nsz],
                ob[:, 0:nsz])

        # filler schedule: (m, half, kc) -> list of thunks
        filler = {}

        def add_filler(m, half, kc, fn):
            filler.setdefault((m, half, kc), []).append(fn)

        # vproj g0 lt4-7: two pre-loop (cover the startup-evac window), two in
        # p0h0; g1 spread over p0h1/p1h0. Projections split q-in-h0 / k-in-h1.
        add_filler(0, 0, -1, lambda: emit_vproj_tile(0, 4))
        add_filler(0, 0, -1, lambda: emit_vproj_tile(0, 5))
        add_filler(0, 0, 4, lambda: emit_vproj_tile(0, 6))
        add_filler(0, 0, 6, lambda: emit_vproj_tile(0, 7))
        add_filler(0, 1, 4, lambda: emit_vproj_tile(1, 0))
        add_filler(0, 1, 8, lambda: emit_vproj_tile(1, 1))
        add_filler(1, 0, 2, lambda: emit_vproj_tile(1, 2))
        add_filler(1, 0, 6, lambda: emit_vproj_tile(1, 3))
        add_filler(1, 1, 4, lambda: emit_vproj_tile(1, 4))
        add_filler(1, 1, 8, lambda: emit_vproj_tile(1, 5))
        add_filler(2, 0, 1, lambda: emit_vproj_tile(1, 6))
        add_filler(2, 0, 5, lambda: emit_vproj_tile(1, 7))
        for m in range(NP - 1):
            add_filler(m, 0, 2, lambda m=m: emit_proj_half(m + 1, "q", 0))
            add_filler(m, 0, 8, lambda m=m: emit_proj_half(m + 1, "q", 1))
            add_filler(m, 1, 2, lambda m=m: emit_proj_half(m + 1, "k", 0))
            add_filler(m, 1, 6, lambda m=m: emit_proj_half(m + 1, "k", 1))
        # wo preload during pair 1
        for p in range(NP):
            add_filler(1, 0, 2 * p + 1, lambda p=p: emit_wo_dma(p))
        # out-proj for query tiles 0-3 inside pair 4 half 1
        for i, (qt, n0, nsz) in enumerate(
                [(qt, n0, nsz) for qt in range(4)
                 for n0, nsz in ((0, 512), (512, 512), (1024, 256))]):
            add_filler(4, 1, i % 10,
                       lambda qt=qt, n0=n0, nsz=nsz: emit_outproj(
                           qt, n0, nsz, early=True))

        # ---- attention main loop ----
        for m in range(NP):
            for half in range(2):
                ctxps = [psc.tile([128, 512], F32, tag="ctx",
                                  name=f"ctx{m}_{half}_{par}")
                         for par in range(2)]
                for fn in filler.get((m, half, -1), ()):
                    fn()
                es_ = {}
                for kc in range(KCH):
                    ss = pss.tile([128, 1024], F32, tag="ss",
                                  name=f"s{m}_{half}_{kc}")
                    for par in range(2):
                        p0 = 64 * par
                        nc.tensor.matmul(
                            ss[:, par * 512:(par + 1) * 512],
                            kts[m][p0:p0 + 64, kc * 128:(kc + 1) * 128],
                            qts[m][p0:p0 + 64,
                                   half * 512:(half + 1) * 512],
                            start=True, stop=True, tile_position=(p0, 0))
                    e = expp.tile([128, 1024], F16, tag="e",
                                  name=f"e{m}_{half}_{kc}")
                    nc.scalar.activation(e[:], ss[:], ActF.Exp, scale=0.125)
                    es_[kc] = e
                    if kc >= 1:
                        ep = es_.pop(kc - 1)
                        for par in range(2):
                            nc.tensor.matmul(
                                ctxps[par][:],
                                v_heads[:, kc - 1, 2 * m + par, :],
                                ep[:, par * 512:(par + 1) * 512],
                                start=(kc - 1 == 0), stop=False)
                    for fn in filler.get((m, half, kc), ()):
                        fn()
                ep = es_.pop(KCH - 1)
                for par in range(2):
                    nc.tensor.matmul(
                        ctxps[par][:],
                        v_heads[:, KCH - 1, 2 * m + par, :],
                        ep[:, par * 512:(par + 1) * 512],
                        start=False, stop=True)
                # normalize: copy denominator rows to SBUF, approx-recip, mul
                if m < NP - 1:
                    for par in range(2):
                        dn = rcp.tile([64, 512], F32, tag="dn",
                                      name=f"dn{m}_{half}_{par}")
                        nc.vector.tensor_copy(dn[:], ctxps[par][64:128, :])
                        rc = rcp.tile([64, 512], F32, tag="rc",
                                      name=f"rc{m}_{half}_{par}")
                        nc.vector.reciprocal_approx_fast(rc[:], dn[:])
                        nc.vector.tensor_mul(
                            ctxT_sb[64 * par:64 * par + 64, m,
                                    half * 512:(half + 1) * 512],
                            ctxps[par][0:64, :], rc[:])
                else:
                    # last pair gates the out-projection: dn copies on the
                    # drained ACT, muls chunked per query tile so each
                    # out-proj qt unblocks as soon as its 128 cols are done
                    rcs = []
                    for par in range(2):
                        dn = rcp.tile([64, 512], F32, tag="dn",
                                      name=f"dn{m}_{half}_{par}")
                        nc.scalar.activation(dn[:], ctxps[par][64:128, :],
                                             ActF.Copy)
                        rc = rcp.tile([64, 512], F32, tag="rc",
                                      name=f"rc{m}_{half}_{par}")
                        nc.vector.reciprocal_approx_fast(rc[:], dn[:])
                        rcs.append(rc)
                    for i in range(4):
                        cs = slice(i * 128, (i + 1) * 128)
                        for par in range(2):
                            nc.vector.tensor_mul(
                                ctxT_sb[64 * par:64 * par + 64, m,
                                        half * 512 + i * 128:
                                        half * 512 + (i + 1) * 128],
                                ctxps[par][0:64, cs], rcs[par][:, cs])

        # ---- output projection, query tiles 4-7 (0-3 emitted above) ----
        for qt in range(4, LT):
            for n0, nsz in ((0, 512), (512, 512), (1024, 256)):
                emit_outproj(qt, n0, nsz)
        at_es.close()
    nc.compile()
    return nc


_NC = None


def _get_nc():
    global _NC
    if _NC is None:
        _NC = _build()
    return _NC


def _prep_in_maps(hidden_states, Wq, Wk, Wv, Wo, K_bg, V_bg):
    hs = np.asarray(hidden_states, np.float32)
    Wq, Wk, Wv, Wo = (np.asarray(w, np.float32) for w in (Wq, Wk, Wv, Wo))
    K_bg = np.asarray(K_bg, np.float32)
    V_bg = np.asarray(V_bg, np.float32)

    hsT = [np.ascontiguousarray(hs[bi].T).astype(np.float16)
           for bi in range(B)]

    def lay_qk(w, g):  # [1280, 640] slice -> [NP, 128, CC, 128]
        sl = w[:, g * 640:(g + 1) * 640]           # [C, 640]
        a = sl.reshape(CC, 128, NP, 128)           # (cc, p, m, n)
        return np.ascontiguousarray(a.transpose(2, 1, 0, 3)).astype(np.float16)

    def lay_wv(w, g):  # [1280, 640] slice -> [2, 128, CC, 320]
        sl = w[:, g * 640:(g + 1) * 640]
        a = sl.reshape(CC, 128, 2, 320)            # (cc, p, gg, n)
        return np.ascontiguousarray(a.transpose(2, 1, 0, 3)).astype(np.float16)

    wq_s = [lay_qk(Wq, g) for g in range(2)]
    wk_s = [lay_qk(Wk, g) for g in range(2)]
    wv_s = [lay_wv(Wv, g) for g in range(2)]
    wo_s = [Wo[g * 640:(g + 1) * 640, :].astype(np.float16) for g in range(2)]

    def pool_bank(x):  # [10, 1024, 64] -> [10, 256, 64], fp16 round + alpha
        x = x.astype(np.float16).astype(np.float32)
        x = x.reshape(HPC, 16, 2, 16, 2, Dh).mean(axis=(2, 4))
        return (ALPHA * x).reshape(HPC, LB, Dh)

    kb_s, vb_s = [], []
    for base in (0, 10, 20, 30):
        kb = pool_bank(K_bg[base:base + HPC])
        vb = pool_bank(V_bg[base:base + HPC])
        kb_s.append(kb.transpose(0, 2, 1).reshape(HPC * Dh, LB).astype(np.float16))
        vb_s.append(vb.transpose(1, 0, 2).reshape(LB, HPC * Dh).astype(np.float16))

    in_maps = []
    for c in range(NCORES):
        bi, g = c // 2, c % 2
        bank = (20 * bi + 10 * g) % 40 // 10
        in_maps.append({
            "hsT": hsT[bi], "wq": wq_s[g], "wk": wk_s[g], "wv": wv_s[g],
            "wo": wo_s[g], "kbT": kb_s[bank], "vb": vb_s[bank],
        })
    return in_maps


def _run(in_maps, **kwargs):
    return bass_utils.run_bass_kernel_spmd(
        _get_nc(), in_maps, core_ids=list(range(NCORES)), **kwargs)


def kernel(hidden_states, Wq, Wk, Wv, Wo, bo, K_bg, V_bg):
    in_maps = _prep_in_maps(hidden_states, Wq, Wk, Wv, Wo, K_bg, V_bg)
    res = _run(in_maps)
    bo = np.asarray(bo, np.float32)
    out = np.empty((B, L, C), np.float32)
    for bi in range(B):
        out[bi] = (res.results[2 * bi]["out"].astype(np.float32)
                   + res.results[2 * bi + 1]["out"].astype(np.float32)
                   + bo[None, :])
    return out
